# revision 12
# baseline (speedup 1.0000x reference)
"""Trainium2 Bass kernel for the HJB loss (nn_HJBLoss_68925635166304).

Reference math (per row b, with Q=diag(1,1,.5,.5), omega=.6, R=.1*I,
G/COV hardcoded, x_target=[1,0,0,0]):

    L_b = a*A + X1*B + X2*C + X3*D + 0.05*(u0^2+u1^2) + 0.25*sigma^2
    a = X0-1
    A = a + 2*X2 + 0.6*u0
    B = X1 + 0.6*X2 + 2*X3 + 0.5*u1
    C = 0.5*X2 + u0 + 0.5*mu0          (folded: (X2*(X2+2*u0+mu0))*0.5)
    D = 0.5*X3 - 0.6*X0 + u1 + 0.5*mu1 (folded: (X3*(X3-1.2*X0+2*u1+mu1))*0.5)
    out = mean_b(L_b)

The a*A term is computed constant-free as sum(X0*E) - sum(X0) - sum(E) + N
with E = X0 + 2*X2 + 0.6*u0; the column sums fall out of the fused
accum_out ports of the scalar_tensor_tensor chain (sum(X0) is recovered
on the host from S1=sum(2*X2+X0), S2=sum(E), S6=sum(2*u0+X2)).

Strategy: pure data parallel over 8 NeuronCores (batch split 8 x 524288).
Per core the shard is laid out [128 partitions x 4096 rows] with each
partition holding a contiguous run of rows (fully contiguous DMAs).
Compute is done on interleaved tiles with strided free-dim views:
fused scalar_tensor_tensor chains + tensor_tensor_reduce products that
reduce straight into per-partition accumulator columns; the u^2/sigma^2
terms ride the scalar engine's Square activation with fused accum_out.
Each core emits a tiny [128, 6*T] accumulator tensor; the host does the
final (exact, float64) sum and divides by B.
"""

import numpy as np

B = 4_194_304
NCORES = 8
R = B // NCORES          # 524288 rows per core
P = 128                  # SBUF partitions
ROWS_PER_LANE = R // P   # 4096
T = 4                    # tiles per core
K = ROWS_PER_LANE // T   # rows per lane per tile
COLS_PER_TILE = 9        # [A, B, C, D, U, S, S1, S2, S6]
ACC_COLS = COLS_PER_TILE * T

_CACHE = {}


def _build(rows=R, tiles=T):
    import concourse.bacc as bacc
    import concourse.mybir as mybir
    from concourse import tile

    f32 = mybir.dt.float32
    Alu = mybir.AluOpType
    Act = mybir.ActivationFunctionType

    T = tiles
    acc_cols = COLS_PER_TILE * T

    # Bacc (not plain Bass): its compile pipeline runs
    # generate_event_semaphores, which splits multi-sem sync waits to
    # satisfy the 1-wait-per-instruction hardware constraint.
    nc = bacc.Bacc(None)
    Xd = nc.declare_dram_parameter("X", [rows, 4], f32, isOutput=False)
    Ud = nc.declare_dram_parameter("u", [rows, 2], f32, isOutput=False)
    Md = nc.declare_dram_parameter("mu", [rows, 2], f32, isOutput=False)
    Sd = nc.declare_dram_parameter("sigma", [rows], f32, isOutput=False)
    Od = nc.declare_dram_parameter("out", [P, acc_cols], f32, isOutput=True)

    Xv = Xd[:].rearrange("(t p k) f -> t p (k f)", t=T, p=P)
    Uv = Ud[:].rearrange("(t p k) f -> t p (k f)", t=T, p=P)
    Mv = Md[:].rearrange("(t p k) f -> t p (k f)", t=T, p=P)
    Sv = Sd[:].rearrange("(t p k) -> t p k", t=T, p=P)
    K = rows // (P * T)

    with tile.TileContext(nc) as tc:
        with (
            tc.tile_pool(name="io", bufs=2) as io,
            tc.tile_pool(name="plane", bufs=8) as plane,
            tc.tile_pool(name="accp", bufs=1) as accp,
        ):
            acc = accp.tile([P, acc_cols], f32)

            for t in range(T):
                base = COLS_PER_TILE * t
                tx = io.tile([P, 4 * K], f32, tag="tx")
                tu = io.tile([P, 2 * K], f32, tag="tu")
                tm = io.tile([P, 2 * K], f32, tag="tm")
                tg = io.tile([P, K], f32, tag="tg")
                nc.sync.dma_start(out=tx[:], in_=Xv[t])
                nc.sync.dma_start(out=tu[:], in_=Uv[t])
                nc.sync.dma_start(out=tm[:], in_=Mv[t])
                nc.sync.dma_start(out=tg[:], in_=Sv[t])

                xv = tx[:].rearrange("p (k f) -> p k f", f=4)
                uv = tu[:].rearrange("p (k f) -> p k f", f=2)
                mv = tm[:].rearrange("p (k f) -> p k f", f=2)
                X0, X1, X2, X3 = (xv[:, :, i] for i in range(4))
                u0, u1 = uv[:, :, 0], uv[:, :, 1]
                m0, m1 = mv[:, :, 0], mv[:, :, 1]

                def stt(out, in0, s, in1, col=None):
                    nc.vector.scalar_tensor_tensor(
                        out=out[:], in0=in0, scalar=float(s), in1=in1,
                        op0=Alu.mult, op1=Alu.add,
                        accum_out=None if col is None
                        else acc[:, base + col:base + col + 1],
                    )

                def ttr(buf, in1, scale, col):
                    # product-with-reduce via TensorScalarPtr: (buf*scale)*in1,
                    # accum_out = sum.  (InstTensorTensorReduce is a custom
                    # DVE op whose uop table isn't loaded under this runtime
                    # -- it crashes the accelerator.)
                    nc.vector.scalar_tensor_tensor(
                        out=buf[:], in0=buf[:], scalar=float(scale), in1=in1,
                        op0=Alu.mult, op1=Alu.mult,
                        accum_out=acc[:, base + col:base + col + 1],
                    )

                # A group: sum((X0-1)*(X0-1+2*X2+0.6*u0)) =
                #   sum(X0*E) - sum(X0) - sum(E) + N,  E = X0+2*X2+0.6*u0
                t1 = plane.tile([P, K], f32, tag="chain")
                stt(t1, X2, 2.0, X0, col=6)        # S1 = sum(2*X2+X0)
                ev = plane.tile([P, K], f32, tag="chain")
                stt(ev, u0, 0.6, t1[:], col=7)     # S2 = sum(E)
                ttr(ev, X0, 1.0, 0)                # A = sum(X0*E)

                # B group: X1 * (X1 + 0.6*X2 + 2*X3 + 0.5*u1)
                b1 = plane.tile([P, K], f32, tag="chain")
                stt(b1, X2, 0.6, X1)
                b2 = plane.tile([P, K], f32, tag="chain")
                stt(b2, X3, 2.0, b1[:])
                bv = plane.tile([P, K], f32, tag="chain")
                stt(bv, u1, 0.5, b2[:])
                ttr(bv, X1, 1.0, 1)

                # C group: 0.5 * X2 * (X2 + 2*u0 + mu0)
                c1 = plane.tile([P, K], f32, tag="chain")
                stt(c1, u0, 2.0, X2, col=8)        # S6 = sum(2*u0+X2)
                cv = plane.tile([P, K], f32, tag="chain")
                stt(cv, m0, 1.0, c1[:])
                ttr(cv, X2, 0.5, 2)

                # D group: 0.5 * X3 * (X3 - 1.2*X0 + 2*u1 + mu1)
                d1 = plane.tile([P, K], f32, tag="chain")
                stt(d1, X0, -1.2, m1)
                d2 = plane.tile([P, K], f32, tag="chain")
                stt(d2, u1, 2.0, d1[:])
                dv = plane.tile([P, K], f32, tag="chain")
                stt(dv, X3, 1.0, d2[:])
                ttr(dv, X3, 0.5, 3)

                # u0^2 + u1^2 and sigma^2 on the scalar engine,
                # host applies the 0.05 / 0.25 weights.
                squ = plane.tile([P, 2 * K], f32, tag="sq")
                nc.scalar.activation(
                    out=squ[:], in_=tu[:], func=Act.Square,
                    accum_out=acc[:, base + 4:base + 5],
                )
                sqg = plane.tile([P, 2 * K], f32, tag="sq")
                nc.scalar.activation(
                    out=sqg[:, 0:K], in_=tg[:], func=Act.Square,
                    accum_out=acc[:, base + 5:base + 6],
                )

            nc.sync.dma_start(out=Od[:], in_=acc[:])

    nc.finalize()
    return nc


def _get_nc():
    if "nc" not in _CACHE:
        _CACHE["nc"] = _build()
    return _CACHE["nc"]


def _run(in_maps, **kwargs):
    from concourse.bass_utils import run_bass_kernel_spmd

    nc = _get_nc()
    return run_bass_kernel_spmd(nc, in_maps, list(range(NCORES)), **kwargs)


def _make_in_maps(X, mu, sigma, u):
    X = np.ascontiguousarray(np.asarray(X, dtype=np.float32))
    mu = np.ascontiguousarray(np.asarray(mu, dtype=np.float32))
    sigma = np.ascontiguousarray(np.asarray(sigma, dtype=np.float32))
    u = np.ascontiguousarray(np.asarray(u, dtype=np.float32))
    maps = []
    for i in range(NCORES):
        sl = slice(i * R, (i + 1) * R)
        maps.append({
            "X": np.ascontiguousarray(X[sl]),
            "u": np.ascontiguousarray(u[sl]),
            "mu": np.ascontiguousarray(mu[sl]),
            "sigma": np.ascontiguousarray(sigma[sl]),
        })
    return maps


def _reduce_outputs(results):
    total = 0.0
    for res in results:
        out = np.asarray(res["out"], dtype=np.float64)  # [P, 9*T]
        c = out.reshape(P, T, COLS_PER_TILE).sum(axis=(0, 1))
        sA, sB, sC, sD, sU, sS, s1, s2, s6 = c
        # Recover column sums: s1=sum(2*X2+X0), s2=sum(E), s6=sum(2*u0+X2)
        sum_u0 = (s2 - s1) / 0.6
        sum_x2 = s6 - 2.0 * sum_u0
        sum_x0 = s1 - 2.0 * sum_x2
        # sum(a*A) = sum(X0*E) - sum(X0) - sum(E) + N
        total += sA - sum_x0 - s2 + R
        total += sB + sC + sD + 0.05 * sU + 0.25 * sS
    return np.float32(total / B)


def bench(in_maps, iters=30, warmup=3):
    """Warm-loop wall timing with device-resident inputs (no per-call H2D).

    Returns (min_s, mean_s) per-call wall time of the 8-core SPMD step.
    """
    import time
    import jax
    import numpy as np_
    from jax.sharding import Mesh, PartitionSpec, NamedSharding
    from jax.experimental.shard_map import shard_map
    from concourse import bass2jax
    from concourse.bass2jax import _bass_exec_p
    import concourse.mybir as mybir

    nc = _get_nc()
    bass2jax.install_neuronx_cc_hook()
    partition_name = nc.partition_id_tensor.name if nc.partition_id_tensor else None
    in_names, out_names, out_avals, zero_outs = [], [], [], []
    for alloc in nc.m.functions[0].allocations:
        if not isinstance(alloc, mybir.MemoryLocationSet):
            continue
        name = alloc.memorylocations[0].name
        if alloc.kind == "ExternalInput":
            if name != partition_name:
                in_names.append(name)
        elif alloc.kind == "ExternalOutput":
            out_names.append(name)
            shape = tuple(alloc.tensor_shape)
            dtype = mybir.dt.np(alloc.dtype)
            out_avals.append(jax.core.ShapedArray(shape, dtype))
            zero_outs.append(np_.zeros(shape, dtype))
    n_params = len(in_names)
    all_in_names = list(in_names) + list(out_names)
    if partition_name is not None:
        all_in_names.append(partition_name)

    def _body(*args):
        operands = list(args)
        if partition_name is not None:
            operands.append(bass2jax.partition_id_tensor())
        outs = _bass_exec_p.bind(
            *operands,
            out_avals=tuple(out_avals),
            in_names=tuple(all_in_names),
            out_names=tuple(out_names),
            lowering_input_output_aliases=(),
            sim_require_finite=True,
            sim_require_nnan=True,
            nc=nc,
        )
        return tuple(outs)

    devices = jax.devices()[:NCORES]
    mesh = Mesh(np_.asarray(devices), ("core",))
    nin = n_params + len(zero_outs)
    fn = jax.jit(
        shard_map(_body, mesh=mesh,
                  in_specs=(PartitionSpec("core"),) * nin,
                  out_specs=(PartitionSpec("core"),) * len(out_names),
                  check_rep=False),
        keep_unused=True,
    )
    sh = NamedSharding(mesh, PartitionSpec("core"))
    concat_in = [
        jax.device_put(
            np_.concatenate([np_.asarray(m[name]) for m in in_maps], axis=0), sh)
        for name in in_names
    ]
    concat_zeros = [
        jax.device_put(
            np_.zeros((NCORES * z.shape[0], *z.shape[1:]), z.dtype), sh)
        for z in zero_outs
    ]
    for _ in range(warmup):
        out = fn(*concat_in, *concat_zeros)
        jax.block_until_ready(out)
    times = []
    for _ in range(iters):
        t0 = time.perf_counter()
        out = fn(*concat_in, *concat_zeros)
        jax.block_until_ready(out)
        times.append(time.perf_counter() - t0)
    return min(times), sum(times) / len(times)


def kernel(X, mu, sigma, u, Q=None, R=None, x_target=None):
    """Full-input entry point: shards across 8 cores, returns scalar mean.

    Q/R/x_target are accepted for signature compatibility; their values are
    hardcoded in the on-device program (they are compile-time constants in
    the reference nn.Module).
    """
    in_maps = _make_in_maps(X, mu, sigma, u)
    res = _run(in_maps)
    return _reduce_outputs(res.results)
